# revision 24
# baseline (speedup 1.0000x reference)
"""Multi-head attention (B=2, S=2048, D=1024, H=16) on 8 trn2 NeuronCores.

Sharding: core c -> batch b = c//4, head-group g = c%4 (4 heads each).
Each core: QKV projections for its 256 output dims, causal attention for its
4 heads, partial output projection over its 256 contraction dims.
Host: sum the 4 partial outputs per batch, add (bo + bv @ wo.T).

Device formulation (per core, all layouts transposed so no P-transpose is
ever needed):
  QT = (wqT_s.T @ xT + bq)         # [256 qdim, 2048 rows] on chip
  KT = same                        # [256, 2048]
  V  = natural [2048 rows, 256] with an appended ones column per head
  sT[k,q] = sum_d KT[d,k] QT[d,q]  -> psum [128k, 512q] tiles
  mask: additive -1e9 on mixed 128x128 blocks (from the real mask input)
  P = exp(sT/8)  (no max subtraction; scores are O(5) so exp is safe and
      softmax is shift-invariant)
  [OT; l] = [V|1].T @ P            # psum [65, 512q]; row 64 = denominator
  OT_norm = OT * (1/l)             # 1/l replicated over partitions via a
                                   # K=2 indicator matmul, then DVE mult
  yT_partial = woT_s.T @ OT_norm   # [1024, 2048] -> DRAM

All PSUM lives in one pool (tags: big/po/ps1 = 4+2+2 banks) so the Tile
scheduler can overlap projections, attention and output projection.
"""

import os
import hashlib
import numpy as np

B, S, D, H, DK = 2, 2048, 1024, 16, 64
NCORES = 8
GROUPS = 4          # head groups per batch
HPG = 4             # heads per group (per core)
GDIM = HPG * DK     # 256 output dims per core
NEG = -1.0e9
QB = 512            # q block width
NQB = S // QB       # 4
NKT = S // 128      # 16 k tiles
NDM = D // 128      # 8 contraction tiles for projections

MODE = os.environ.get("BASS_MHA_MODE", "bf16")  # fp32 | bf16

_CACHE = {}


def _make_plan(m2d):
    """Classify 128x128 blocks of the (q,k) mask into skip/full/mixed.

    Returns per (qb, j): (j, cmin_local, bias_cols) where bias_cols is a list
    of (c_local, uniq_tile_idx); plus the packed unique bias blocks.
    """
    sub = np.asarray(m2d).reshape(S // 128, 128, S // 128, 128)
    any_ = sub.any(axis=(1, 3))   # [qtile, ktile]
    all_ = sub.all(axis=(1, 3))

    uniq = {}
    uniq_src = []
    plan = []
    for qb in range(NQB):
        entries = []
        cs = list(range(4 * qb, 4 * qb + 4))
        for j in range(NKT):
            states = []
            for c in cs:
                if not any_[c, j]:
                    states.append("skip")
                elif all_[c, j]:
                    states.append("full")
                else:
                    states.append("mixed")
            if all(s == "skip" for s in states):
                continue
            cmin = next(i for i, s in enumerate(states) if s != "skip")
            bias_cols = []
            for i in range(cmin, 4):
                if states[i] == "full":
                    continue
                c = cs[i]
                if states[i] == "skip":
                    blk = np.full((128, 128), NEG, np.float32)
                else:
                    m = sub[c, :, j, :]  # [128 q, 128 k]
                    blk = np.where(m.T != 0, 0.0, NEG).astype(np.float32)
                tri = False  # gpsimd affine_select path regressed; DVE adds
                if tri:
                    # canonical causal diagonal: zero k>q after the exp via
                    # gpsimd affine_select, no bias tile needed
                    bias_cols.append((i, -1))
                    continue
                hsh = hashlib.sha1(blk.tobytes()).hexdigest()
                if hsh not in uniq:
                    uniq[hsh] = len(uniq_src)
                    uniq_src.append(blk)
                bias_cols.append((i, uniq[hsh]))
            entries.append((j, cmin, bias_cols))
        plan.append(entries)
    bias_pack = (
        np.stack(uniq_src) if uniq_src else np.zeros((1, 128, 128), np.float32)
    )
    key = hashlib.sha1(
        repr([(qb, e) for qb, e in enumerate(plan)]).encode()
    ).hexdigest()
    return plan, bias_pack, key


def _build(mode, plan, n_bias):
    import concourse.mybir as mybir
    from concourse import bacc, tile

    f32 = mybir.dt.float32
    bf16 = mybir.dt.bfloat16
    st_dt = bf16 if mode == "bf16" else f32

    AF = mybir.ActivationFunctionType
    AO = mybir.AluOpType

    nc = bacc.Bacc(
        "TRN2", target_bir_lowering=False, debug=False, num_devices=NCORES
    )

    io_dt = bf16 if mode == "bf16" else f32
    qT_d = nc.declare_dram_parameter("qT", [D, S], io_dt, isOutput=False).ap()
    kT_d = nc.declare_dram_parameter("kT", [D, S], io_dt, isOutput=False).ap()
    vT_d = nc.declare_dram_parameter("vT", [D, S], io_dt, isOutput=False).ap()
    wqT_d = nc.declare_dram_parameter("wqT", [D, GDIM], io_dt, isOutput=False).ap()
    wkT_d = nc.declare_dram_parameter("wkT", [D, GDIM], io_dt, isOutput=False).ap()
    wvT_d = nc.declare_dram_parameter("wvT", [D, GDIM], io_dt, isOutput=False).ap()
    woT_d = nc.declare_dram_parameter("woT", [GDIM, D], io_dt, isOutput=False).ap()
    bq_d = nc.declare_dram_parameter("bq2", [128, 2], f32, isOutput=False).ap()
    bk_d = nc.declare_dram_parameter("bk2", [128, 2], f32, isOutput=False).ap()
    bias_d = nc.declare_dram_parameter(
        "bias_pack", [n_bias, 128, 128], f32, isOutput=False
    ).ap()
    yT_d = nc.declare_dram_parameter("yT", [D, S], f32, isOutput=True).ap()

    with tile.TileContext(nc) as tc:
        with (
            tc.tile_pool(name="res", bufs=1) as res,
            tc.tile_pool(name="ot_pool", bufs=2) as ot_pool,
            tc.tile_pool(name="instream", bufs=8) as instream,
            tc.tile_pool(name="ptp", bufs=4) as ptp,
            tc.tile_pool(name="ystage", bufs=3) as ystage,
            tc.tile_pool(name="small", bufs=4) as small,
            tc.tile_pool(name="psum", bufs=2, space="PSUM") as psum,
        ):
            # ---- resident weights / constants ----
            dma = nc.sync.dma_start

            wq_sb = res.tile([128, NDM, GDIM], st_dt, name="wq_sb")
            dma(out=wq_sb, in_=wqT_d.rearrange("(dm p) o -> p dm o", p=128))
            wk_sb = res.tile([128, NDM, GDIM], st_dt, name="wk_sb")
            dma(out=wk_sb, in_=wkT_d.rearrange("(dm p) o -> p dm o", p=128))
            wv_sb = res.tile([128, NDM, GDIM], st_dt, name="wv_sb")
            dma(out=wv_sb, in_=wvT_d.rearrange("(dm p) o -> p dm o", p=128))
            wo_sb = res.tile([128, 2, D], st_dt, name="wo_sb")
            dma(out=wo_sb, in_=woT_d.rearrange("(ct p) o -> p ct o", p=128))
            bq_sb = res.tile([128, 2], f32, name="bq_sb")
            dma(out=bq_sb, in_=bq_d)
            bk_sb = res.tile([128, 2], f32, name="bk_sb")
            dma(out=bk_sb, in_=bk_d)
            bias_sb = res.tile([128, n_bias, 128], f32, name="bias_sb")
            dma(out=bias_sb, in_=bias_d.rearrange("n p o -> p n o"))

            # per-chunk tiles so chunk-level deps stay exact (lets attention
            # on q-block i overlap projections of chunk i+1)
            QT_c = [res.tile([128, 2, 512], st_dt, name=f"QT{i}") for i in range(4)]
            KT_c = [res.tile([128, 2, 512], st_dt, name=f"KT{i}") for i in range(4)]
            V_c = [
                res.tile([128, 4, HPG, DK + 1], st_dt, name=f"V{i}")
                for i in range(4)
            ]
            for i in range(4):
                nc.vector.memset(V_c[i][:, :, :, DK : DK + 1], 1.0)
            ones64_sb = res.tile([1, 64], f32, name="ones64_sb")
            nc.vector.memset(ones64_sb, 1.0)

            def proj_chunk(ci):
                # Q and K projection for 512-row chunk ci
                for src_d, w_sb, b_sb, dst in (
                    (qT_d, wq_sb, bq_sb, QT_c[ci]),
                    (kT_d, wk_sb, bk_sb, KT_c[ci]),
                ):
                    ps_p = psum.tile([128, 2, 512], f32, name="ps_p", tag="big")
                    for dm in range(NDM):
                        xt = instream.tile([128, 512], st_dt, name="xt", tag="xt")
                        dma(
                            out=xt,
                            in_=src_d[
                                128 * dm : 128 * (dm + 1),
                                512 * ci : 512 * (ci + 1),
                            ],
                        )
                        for ot in range(2):
                            nc.tensor.matmul(
                                ps_p[:, ot, :],
                                lhsT=w_sb[:, dm, 128 * ot : 128 * (ot + 1)],
                                rhs=xt,
                                start=(dm == 0),
                                stop=(dm == NDM - 1),
                            )
                    for ot in range(2):
                        nc.vector.tensor_scalar_add(
                            dst[:, ot, :], ps_p[:, ot, :], b_sb[:, ot : ot + 1]
                        )
                # V projection for the same rows
                vts = []
                for dm in range(NDM):
                    vt = instream.tile(
                        [128, 512], st_dt, name="vt", tag="vt", bufs=10
                    )
                    dma(
                        out=vt,
                        in_=vT_d[
                            128 * dm : 128 * (dm + 1),
                            512 * ci : 512 * (ci + 1),
                        ],
                    )
                    vts.append(vt)
                for half in range(2):
                    ps_v = psum.tile([128, 2, 512], f32, name="ps_v", tag="big")
                    for dm in range(NDM):
                        for rl in range(2):
                            rt = 2 * half + rl
                            nc.tensor.matmul(
                                ps_v[:, rl, 0:GDIM],
                                lhsT=vts[dm][:, 128 * rt : 128 * (rt + 1)],
                                rhs=wv_sb[:, dm, :],
                                start=(dm == 0),
                                stop=(dm == NDM - 1),
                            )
                    for rl in range(2):
                        nc.vector.tensor_copy(
                            out=V_c[ci][:, 2 * half + rl, :, 0:DK],
                            in_=ps_v[:, rl, 0:GDIM].rearrange(
                                "p (h d) -> p h d", d=DK
                            ),
                        )

            def attn_qb(qb):
                entries = plan[qb]
                OT_sb = ot_pool.tile([128, 2, QB], st_dt, name="OT_sb")
                last_j = entries[-1][0]
                first_j = entries[0][0]
                for pr in range(2):
                    heads = (2 * pr, 2 * pr + 1)
                    po = {}
                    for h in heads:
                        po[h] = psum.tile(
                            [DK + 1, QB], f32, name=f"po{h}", tag="po", bufs=3
                        )
                    for j, cmin, bias_cols in entries:
                        off = 128 * cmin
                        jc, jl = j // 4, j % 4
                        ps_s = psum.tile(
                            [128, 2, QB], f32, name="ps_s", tag="big"
                        )
                        for hh, h in enumerate(heads):
                            p0 = 64 * hh
                            ht = h // 2
                            nc.tensor.matmul(
                                ps_s[:, hh, off:QB],
                                lhsT=KT_c[jc][
                                    p0 : p0 + 64, ht, 128 * jl : 128 * (jl + 1)
                                ],
                                rhs=QT_c[qb][p0 : p0 + 64, ht, off:QB],
                                start=True,
                                stop=True,
                            )
                        for hh in range(2):
                            for cl, ui in bias_cols:
                                co = 128 * cl
                                nc.vector.tensor_tensor(
                                    out=ps_s[:, hh, co : co + 128],
                                    in0=ps_s[:, hh, co : co + 128],
                                    in1=bias_sb[:, ui, :],
                                    op=AO.add,
                                )
                        pt = ptp.tile([128, 2, QB], st_dt, name="pt")
                        nc.scalar.activation(
                            pt[:, :, off:QB],
                            ps_s[:, :, off:QB],
                            AF.Exp,
                            scale=0.125,
                        )
                        for hh, h in enumerate(heads):
                            nc.tensor.matmul(
                                po[h][:, off:QB],
                                lhsT=V_c[jc][:, jl, h, :],
                                rhs=pt[:, hh, off:QB],
                                start=(j == first_j),
                                stop=(j == last_j),
                            )
                    # normalize: 1/l per head, replicate across 64 partitions
                    ps_rl = psum.tile(
                        [128, QB], f32, name="ps_rl", tag="ps1", bufs=1
                    )
                    for hh, h in enumerate(heads):
                        l1 = small.tile([1, QB], f32, name="l1", tag=f"l{hh}")
                        nc.scalar.activation(
                            l1, po[h][DK : DK + 1, :], AF.Copy
                        )
                        rl1 = small.tile(
                            [1, QB], f32, name="rl1", tag=f"rl{hh}"
                        )
                        nc.vector.reciprocal_approx_fast(out=rl1, in_=l1)
                        nc.tensor.matmul(
                            ps_rl[64 * hh : 64 * hh + 64, :],
                            lhsT=ones64_sb,
                            rhs=rl1,
                            start=True,
                            stop=True,
                        )
                    rl_bc = small.tile(
                        [128, QB], f32, name="rl_bc", tag="rl_bc"
                    )
                    nc.scalar.activation(rl_bc, ps_rl, AF.Copy)
                    for hh, h in enumerate(heads):
                        p0 = 64 * hh
                        nc.vector.tensor_tensor(
                            out=OT_sb[p0 : p0 + 64, h // 2, :],
                            in0=po[h][0:DK, :],
                            in1=rl_bc[p0 : p0 + 64, :],
                            op=AO.mult,
                        )
                for ot8 in range(8):
                    ps_y = psum.tile(
                        [128, QB], f32, name="ps_y", tag="ps1", bufs=1
                    )
                    for ct in range(2):
                        nc.tensor.matmul(
                            ps_y,
                            lhsT=wo_sb[:, ct, 128 * ot8 : 128 * (ot8 + 1)],
                            rhs=OT_sb[:, ct, :],
                            start=(ct == 0),
                            stop=(ct == 1),
                        )
                    ysb = ystage.tile([128, QB], f32, name="ysb")
                    nc.vector.tensor_copy(out=ysb, in_=ps_y)
                    dma(
                        out=yT_d[
                            128 * ot8 : 128 * (ot8 + 1),
                            QB * qb : QB * (qb + 1),
                        ],
                        in_=ysb,
                    )

            # interleave: projection chunk ci, then attention q-block ci
            # (A(qb) needs exactly chunks <= qb of K/V and chunk qb of Q)
            for ci in range(4):
                proj_chunk(ci)
                attn_qb(ci)

    nc.compile()
    return nc


def _get_nc(mode, plan, n_bias, key):
    ck = (mode, key, n_bias)
    if ck not in _CACHE:
        _CACHE[ck] = _build(mode, plan, n_bias)
    return _CACHE[ck]


def _prep_inputs(q, k, v, wq, bq, wk, bk, wv, wo, bias_pack, mode):
    """Build the 8 per-core input maps."""
    f32 = np.float32
    if mode == "bf16":
        import ml_dtypes

        io_np = ml_dtypes.bfloat16
    else:
        io_np = f32

    wqT = np.ascontiguousarray(np.asarray(wq, f32).T)
    wkT = np.ascontiguousarray(np.asarray(wk, f32).T)
    wvT = np.ascontiguousarray(np.asarray(wv, f32).T)
    woT = np.ascontiguousarray(np.asarray(wo, f32).T)

    in_maps = []
    for c in range(NCORES):
        b, g = c // GROUPS, c % GROUPS
        sl = slice(GDIM * g, GDIM * (g + 1))
        im = {
            "qT": np.ascontiguousarray(np.asarray(q[b], f32).T).astype(io_np),
            "kT": np.ascontiguousarray(np.asarray(k[b], f32).T).astype(io_np),
            "vT": np.ascontiguousarray(np.asarray(v[b], f32).T).astype(io_np),
            "wqT": np.ascontiguousarray(wqT[:, sl]).astype(io_np),
            "wkT": np.ascontiguousarray(wkT[:, sl]).astype(io_np),
            "wvT": np.ascontiguousarray(wvT[:, sl]).astype(io_np),
            "woT": np.ascontiguousarray(woT[sl, :]).astype(io_np),
            "bq2": np.ascontiguousarray(
                np.asarray(bq, f32)[sl].reshape(2, 128).T
            ),
            "bk2": np.ascontiguousarray(
                np.asarray(bk, f32)[sl].reshape(2, 128).T
            ),
            "bias_pack": bias_pack,
        }
        in_maps.append(im)
    return in_maps


def _kernel_impl(q, k, v, mask, wq, bq, wk, bk, wv, bv, wo, bo, trace=False):
    from concourse.bass_utils import run_bass_kernel_spmd

    f32 = np.float32
    m2d = np.asarray(mask)[0, 0]
    plan, bias_pack, key = _make_plan(m2d)
    nc = _get_nc(MODE, plan, bias_pack.shape[0], key)
    in_maps = _prep_inputs(q, k, v, wq, bq, wk, bk, wv, wo, bias_pack, MODE)

    res = run_bass_kernel_spmd(nc, in_maps, list(range(NCORES)), trace=trace)

    bo_eff = (
        np.asarray(bo, np.float64)
        + np.asarray(bv, np.float64) @ np.asarray(wo, np.float64).T
    ).astype(f32)

    out = np.zeros((B, S, D), f32)
    for c in range(NCORES):
        out[c // GROUPS] += res.results[c]["yT"].T
    out += bo_eff
    return out, res


def kernel(q, k, v, mask, wq, bq, wk, bk, wv, bv, wo, bo):
    out, _ = _kernel_impl(q, k, v, mask, wq, bq, wk, bk, wv, bv, wo, bo)
    return out


# revision 26
# speedup vs baseline: 1.0237x; 1.0237x over previous
"""Multi-head attention (B=2, S=2048, D=1024, H=16) on 8 trn2 NeuronCores.

Sharding: core c -> batch b = c//4, head-group g = c%4 (4 heads each).
Each core: QKV projections for its 256 output dims, causal attention for its
4 heads, partial output projection over its 256 contraction dims.
Host: sum the 4 partial outputs per batch, add (bo + bv @ wo.T).

Device formulation (per core, all layouts transposed so no P-transpose is
ever needed):
  QT = (wqT_s.T @ xT + bq)         # [256 qdim, 2048 rows] on chip
  KT = same                        # [256, 2048]
  V  = natural [2048 rows, 256] with an appended ones column per head
  sT[k,q] = sum_d KT[d,k] QT[d,q]  -> psum [128k, 512q] tiles
  mask: additive -1e9 on mixed 128x128 blocks (from the real mask input)
  P = exp(sT/8)  (no max subtraction; scores are O(5) so exp is safe and
      softmax is shift-invariant)
  [OT; l] = [V|1].T @ P            # psum [65, 512q]; row 64 = denominator
  OT_norm = OT * (1/l)             # 1/l replicated over partitions via a
                                   # K=2 indicator matmul, then DVE mult
  yT_partial = woT_s.T @ OT_norm   # [1024, 2048] -> DRAM

All PSUM lives in one pool (tags: big/po/ps1 = 4+2+2 banks) so the Tile
scheduler can overlap projections, attention and output projection.
"""

import os
import hashlib
import numpy as np

B, S, D, H, DK = 2, 2048, 1024, 16, 64
NCORES = 8
GROUPS = 4          # head groups per batch
HPG = 4             # heads per group (per core)
GDIM = HPG * DK     # 256 output dims per core
NEG = -1.0e9
QB = 512            # q block width
NQB = S // QB       # 4
NKT = S // 128      # 16 k tiles
NDM = D // 128      # 8 contraction tiles for projections

MODE = os.environ.get("BASS_MHA_MODE", "bf16")  # fp32 | bf16

_CACHE = {}


def _make_plan(m2d):
    """Classify 128x128 blocks of the (q,k) mask into skip/full/mixed.

    Returns per (qb, j): (j, cmin_local, bias_cols) where bias_cols is a list
    of (c_local, uniq_tile_idx); plus the packed unique bias blocks.
    """
    sub = np.asarray(m2d).reshape(S // 128, 128, S // 128, 128)
    any_ = sub.any(axis=(1, 3))   # [qtile, ktile]
    all_ = sub.all(axis=(1, 3))

    uniq = {}
    uniq_src = []
    plan = []
    for qb in range(NQB):
        entries = []
        cs = list(range(4 * qb, 4 * qb + 4))
        for j in range(NKT):
            states = []
            for c in cs:
                if not any_[c, j]:
                    states.append("skip")
                elif all_[c, j]:
                    states.append("full")
                else:
                    states.append("mixed")
            if all(s == "skip" for s in states):
                continue
            cmin = next(i for i, s in enumerate(states) if s != "skip")
            bias_cols = []
            for i in range(cmin, 4):
                if states[i] == "full":
                    continue
                c = cs[i]
                if states[i] == "skip":
                    blk = np.full((128, 128), NEG, np.float32)
                else:
                    m = sub[c, :, j, :]  # [128 q, 128 k]
                    blk = np.where(m.T != 0, 0.0, NEG).astype(np.float32)
                tri = False  # gpsimd affine_select path regressed; DVE adds
                if tri:
                    # canonical causal diagonal: zero k>q after the exp via
                    # gpsimd affine_select, no bias tile needed
                    bias_cols.append((i, -1))
                    continue
                hsh = hashlib.sha1(blk.tobytes()).hexdigest()
                if hsh not in uniq:
                    uniq[hsh] = len(uniq_src)
                    uniq_src.append(blk)
                bias_cols.append((i, uniq[hsh]))
            entries.append((j, cmin, bias_cols))
        plan.append(entries)
    bias_pack = (
        np.stack(uniq_src) if uniq_src else np.zeros((1, 128, 128), np.float32)
    )
    key = hashlib.sha1(
        repr([(qb, e) for qb, e in enumerate(plan)]).encode()
    ).hexdigest()
    return plan, bias_pack, key


def _build(mode, plan, n_bias):
    import concourse.mybir as mybir
    from concourse import bacc, tile

    f32 = mybir.dt.float32
    bf16 = mybir.dt.bfloat16
    st_dt = bf16 if mode == "bf16" else f32

    AF = mybir.ActivationFunctionType
    AO = mybir.AluOpType

    nc = bacc.Bacc(
        "TRN2", target_bir_lowering=False, debug=False, num_devices=NCORES
    )

    io_dt = bf16 if mode == "bf16" else f32
    qT_d = nc.declare_dram_parameter("qT", [D, S], io_dt, isOutput=False).ap()
    kT_d = nc.declare_dram_parameter("kT", [D, S], io_dt, isOutput=False).ap()
    vT_d = nc.declare_dram_parameter("vT", [D, S], io_dt, isOutput=False).ap()
    wqT_d = nc.declare_dram_parameter("wqT", [D, GDIM], io_dt, isOutput=False).ap()
    wkT_d = nc.declare_dram_parameter("wkT", [D, GDIM], io_dt, isOutput=False).ap()
    wvT_d = nc.declare_dram_parameter("wvT", [D, GDIM], io_dt, isOutput=False).ap()
    woT_d = nc.declare_dram_parameter("woT", [GDIM, D], io_dt, isOutput=False).ap()
    bq_d = nc.declare_dram_parameter("bq2", [128, 2], f32, isOutput=False).ap()
    bk_d = nc.declare_dram_parameter("bk2", [128, 2], f32, isOutput=False).ap()
    bias_d = nc.declare_dram_parameter(
        "bias_pack", [n_bias, 128, 128], f32, isOutput=False
    ).ap()
    yT_d = nc.declare_dram_parameter("yT", [D, S], f32, isOutput=True).ap()

    with tile.TileContext(nc) as tc:
        with (
            tc.tile_pool(name="res", bufs=1) as res,
            tc.tile_pool(name="ot_pool", bufs=2) as ot_pool,
            tc.tile_pool(name="instream", bufs=8) as instream,
            tc.tile_pool(name="ptp", bufs=4) as ptp,
            tc.tile_pool(name="ystage", bufs=3) as ystage,
            tc.tile_pool(name="small", bufs=4) as small,
            tc.tile_pool(name="psum", bufs=2, space="PSUM") as psum,
        ):
            # ---- resident weights / constants ----
            dma = nc.sync.dma_start

            wq_sb = res.tile([128, NDM, GDIM], st_dt, name="wq_sb")
            dma(out=wq_sb, in_=wqT_d.rearrange("(dm p) o -> p dm o", p=128))
            wk_sb = res.tile([128, NDM, GDIM], st_dt, name="wk_sb")
            dma(out=wk_sb, in_=wkT_d.rearrange("(dm p) o -> p dm o", p=128))
            wv_sb = res.tile([128, NDM, GDIM], st_dt, name="wv_sb")
            dma(out=wv_sb, in_=wvT_d.rearrange("(dm p) o -> p dm o", p=128))
            wo_sb = res.tile([128, 2, D], st_dt, name="wo_sb")
            dma(out=wo_sb, in_=woT_d.rearrange("(ct p) o -> p ct o", p=128))
            bq_sb = res.tile([128, 2], f32, name="bq_sb")
            dma(out=bq_sb, in_=bq_d)
            bk_sb = res.tile([128, 2], f32, name="bk_sb")
            dma(out=bk_sb, in_=bk_d)
            bias_sb = res.tile([128, n_bias, 128], f32, name="bias_sb")
            dma(out=bias_sb, in_=bias_d.rearrange("n p o -> p n o"))

            # per-chunk tiles so chunk-level deps stay exact (lets attention
            # on q-block i overlap projections of chunk i+1)
            QT_c = [res.tile([128, 2, 512], st_dt, name=f"QT{i}") for i in range(4)]
            KT_c = [res.tile([128, 2, 512], st_dt, name=f"KT{i}") for i in range(4)]
            V_c = [
                res.tile([128, 4, HPG, DK + 1], st_dt, name=f"V{i}")
                for i in range(4)
            ]
            for i in range(4):
                nc.vector.memset(V_c[i][:, :, :, DK : DK + 1], 1.0)
            ones64_sb = res.tile([1, 64], f32, name="ones64_sb")
            nc.vector.memset(ones64_sb, 1.0)

            def proj_chunk(ci):
                # Q and K projection for 512-row chunk ci
                for src_d, w_sb, b_sb, dst in (
                    (qT_d, wq_sb, bq_sb, QT_c[ci]),
                    (kT_d, wk_sb, bk_sb, KT_c[ci]),
                ):
                    ps_p = psum.tile([128, 2, 512], f32, name="ps_p", tag="big")
                    for dm in range(NDM):
                        xt = instream.tile([128, 512], st_dt, name="xt", tag="xt")
                        dma(
                            out=xt,
                            in_=src_d[
                                128 * dm : 128 * (dm + 1),
                                512 * ci : 512 * (ci + 1),
                            ],
                        )
                        for ot in range(2):
                            nc.tensor.matmul(
                                ps_p[:, ot, :],
                                lhsT=w_sb[:, dm, 128 * ot : 128 * (ot + 1)],
                                rhs=xt,
                                start=(dm == 0),
                                stop=(dm == NDM - 1),
                            )
                    for ot in range(2):
                        nc.vector.tensor_scalar_add(
                            dst[:, ot, :], ps_p[:, ot, :], b_sb[:, ot : ot + 1]
                        )
                # V projection for the same rows
                vts = []
                for dm in range(NDM):
                    vt = instream.tile(
                        [128, 512], st_dt, name="vt", tag="vt", bufs=10
                    )
                    dma(
                        out=vt,
                        in_=vT_d[
                            128 * dm : 128 * (dm + 1),
                            512 * ci : 512 * (ci + 1),
                        ],
                    )
                    vts.append(vt)
                for half in range(2):
                    ps_v = psum.tile([128, 2, 512], f32, name="ps_v", tag="big")
                    for dm in range(NDM):
                        for rl in range(2):
                            rt = 2 * half + rl
                            nc.tensor.matmul(
                                ps_v[:, rl, 0:GDIM],
                                lhsT=vts[dm][:, 128 * rt : 128 * (rt + 1)],
                                rhs=wv_sb[:, dm, :],
                                start=(dm == 0),
                                stop=(dm == NDM - 1),
                            )
                    for rl in range(2):
                        nc.vector.tensor_copy(
                            out=V_c[ci][:, 2 * half + rl, :, 0:DK],
                            in_=ps_v[:, rl, 0:GDIM].rearrange(
                                "p (h d) -> p h d", d=DK
                            ),
                        )

            def attn_pair(qb, pr, OT_sb):
                entries = plan[qb]
                last_j = entries[-1][0]
                first_j = entries[0][0]
                heads = (2 * pr, 2 * pr + 1)
                po = {}
                for h in heads:
                    po[h] = psum.tile(
                        [DK + 1, QB], f32, name=f"po{h}", tag="po", bufs=3
                    )
                for j, cmin, bias_cols in entries:
                    off = 128 * cmin
                    jc, jl = j // 4, j % 4
                    ps_s = psum.tile(
                        [128, 2, QB], f32, name="ps_s", tag="big"
                    )
                    for hh, h in enumerate(heads):
                        p0 = 64 * hh
                        ht = h // 2
                        nc.tensor.matmul(
                            ps_s[:, hh, off:QB],
                            lhsT=KT_c[jc][
                                p0 : p0 + 64, ht, 128 * jl : 128 * (jl + 1)
                            ],
                            rhs=QT_c[qb][p0 : p0 + 64, ht, off:QB],
                            start=True,
                            stop=True,
                        )
                    for hh in range(2):
                        for cl, ui in bias_cols:
                            co = 128 * cl
                            nc.vector.tensor_tensor(
                                out=ps_s[:, hh, co : co + 128],
                                in0=ps_s[:, hh, co : co + 128],
                                in1=bias_sb[:, ui, :],
                                op=AO.add,
                            )
                    pt = ptp.tile([128, 2, QB], st_dt, name="pt")
                    nc.scalar.activation(
                        pt[:, :, off:QB],
                        ps_s[:, :, off:QB],
                        AF.Exp,
                        scale=0.125,
                    )
                    for hh, h in enumerate(heads):
                        nc.tensor.matmul(
                            po[h][:, off:QB],
                            lhsT=V_c[jc][:, jl, h, :],
                            rhs=pt[:, hh, off:QB],
                            start=(j == first_j),
                            stop=(j == last_j),
                        )
                # normalize: 1/l per head, replicate across 64 partitions
                ps_rl = psum.tile(
                    [128, QB], f32, name="ps_rl", tag="ps1", bufs=1
                )
                for hh, h in enumerate(heads):
                    l1 = small.tile([1, QB], f32, name="l1", tag=f"l{hh}")
                    nc.scalar.activation(
                        l1, po[h][DK : DK + 1, :], AF.Copy
                    )
                    rl1 = small.tile(
                        [1, QB], f32, name="rl1", tag=f"rl{hh}"
                    )
                    nc.vector.reciprocal_approx_fast(out=rl1, in_=l1)
                    nc.tensor.matmul(
                        ps_rl[64 * hh : 64 * hh + 64, :],
                        lhsT=ones64_sb,
                        rhs=rl1,
                        start=True,
                        stop=True,
                    )
                rl_bc = small.tile(
                    [128, QB], f32, name="rl_bc", tag="rl_bc"
                )
                nc.scalar.activation(rl_bc, ps_rl, AF.Copy)
                for hh, h in enumerate(heads):
                    p0 = 64 * hh
                    nc.vector.tensor_tensor(
                        out=OT_sb[p0 : p0 + 64, h // 2, :],
                        in0=po[h][0:DK, :],
                        in1=rl_bc[p0 : p0 + 64, :],
                        op=AO.mult,
                    )

            def outproj(qb, OT_sb):
                for ot8 in range(8):
                    ps_y = psum.tile(
                        [128, QB], f32, name="ps_y", tag="ps1", bufs=1
                    )
                    for ct in range(2):
                        nc.tensor.matmul(
                            ps_y,
                            lhsT=wo_sb[:, ct, 128 * ot8 : 128 * (ot8 + 1)],
                            rhs=OT_sb[:, ct, :],
                            start=(ct == 0),
                            stop=(ct == 1),
                        )
                    ysb = ystage.tile([128, QB], f32, name="ysb")
                    nc.vector.tensor_copy(out=ysb, in_=ps_y)
                    dma(
                        out=yT_d[
                            128 * ot8 : 128 * (ot8 + 1),
                            QB * qb : QB * (qb + 1),
                        ],
                        in_=ysb,
                    )

            # projections interleaved with attention; the final pairs of
            # q-blocks 2 and 3 are interleaved so the tail has two
            # independent dependency chains for the scheduler to overlap
            OTm = {}
            for qb in range(4):
                OTm[qb] = ot_pool.tile(
                    [128, 2, QB], st_dt, name=f"OT{qb}", tag="OT", bufs=2
                )
            proj_chunk(0)
            attn_pair(0, 0, OTm[0])
            attn_pair(0, 1, OTm[0])
            outproj(0, OTm[0])
            proj_chunk(1)
            attn_pair(1, 0, OTm[1])
            attn_pair(1, 1, OTm[1])
            outproj(1, OTm[1])
            proj_chunk(2)
            attn_pair(2, 0, OTm[2])
            proj_chunk(3)
            attn_pair(3, 0, OTm[3])
            attn_pair(2, 1, OTm[2])
            outproj(2, OTm[2])
            attn_pair(3, 1, OTm[3])
            outproj(3, OTm[3])


    nc.compile()
    return nc


def _get_nc(mode, plan, n_bias, key):
    ck = (mode, key, n_bias)
    if ck not in _CACHE:
        _CACHE[ck] = _build(mode, plan, n_bias)
    return _CACHE[ck]


def _prep_inputs(q, k, v, wq, bq, wk, bk, wv, wo, bias_pack, mode):
    """Build the 8 per-core input maps."""
    f32 = np.float32
    if mode == "bf16":
        import ml_dtypes

        io_np = ml_dtypes.bfloat16
    else:
        io_np = f32

    wqT = np.ascontiguousarray(np.asarray(wq, f32).T)
    wkT = np.ascontiguousarray(np.asarray(wk, f32).T)
    wvT = np.ascontiguousarray(np.asarray(wv, f32).T)
    woT = np.ascontiguousarray(np.asarray(wo, f32).T)

    in_maps = []
    for c in range(NCORES):
        b, g = c // GROUPS, c % GROUPS
        sl = slice(GDIM * g, GDIM * (g + 1))
        im = {
            "qT": np.ascontiguousarray(np.asarray(q[b], f32).T).astype(io_np),
            "kT": np.ascontiguousarray(np.asarray(k[b], f32).T).astype(io_np),
            "vT": np.ascontiguousarray(np.asarray(v[b], f32).T).astype(io_np),
            "wqT": np.ascontiguousarray(wqT[:, sl]).astype(io_np),
            "wkT": np.ascontiguousarray(wkT[:, sl]).astype(io_np),
            "wvT": np.ascontiguousarray(wvT[:, sl]).astype(io_np),
            "woT": np.ascontiguousarray(woT[sl, :]).astype(io_np),
            "bq2": np.ascontiguousarray(
                np.asarray(bq, f32)[sl].reshape(2, 128).T
            ),
            "bk2": np.ascontiguousarray(
                np.asarray(bk, f32)[sl].reshape(2, 128).T
            ),
            "bias_pack": bias_pack,
        }
        in_maps.append(im)
    return in_maps


def _kernel_impl(q, k, v, mask, wq, bq, wk, bk, wv, bv, wo, bo, trace=False):
    from concourse.bass_utils import run_bass_kernel_spmd

    f32 = np.float32
    m2d = np.asarray(mask)[0, 0]
    plan, bias_pack, key = _make_plan(m2d)
    nc = _get_nc(MODE, plan, bias_pack.shape[0], key)
    in_maps = _prep_inputs(q, k, v, wq, bq, wk, bk, wv, wo, bias_pack, MODE)

    res = run_bass_kernel_spmd(nc, in_maps, list(range(NCORES)), trace=trace)

    bo_eff = (
        np.asarray(bo, np.float64)
        + np.asarray(bv, np.float64) @ np.asarray(wo, np.float64).T
    ).astype(f32)

    out = np.zeros((B, S, D), f32)
    for c in range(NCORES):
        out[c // GROUPS] += res.results[c]["yT"].T
    out += bo_eff
    return out, res


def kernel(q, k, v, mask, wq, bq, wk, bk, wv, bv, wo, bo):
    out, _ = _kernel_impl(q, k, v, mask, wq, bq, wk, bk, wv, bv, wo, bo)
    return out


# revision 27
# speedup vs baseline: 1.0578x; 1.0332x over previous
"""Multi-head attention (B=2, S=2048, D=1024, H=16) on 8 trn2 NeuronCores.

Sharding: core c -> batch b = c//4, head-group g = c%4 (4 heads each).
Each core: QKV projections for its 256 output dims, causal attention for its
4 heads, partial output projection over its 256 contraction dims.
Host: sum the 4 partial outputs per batch, add (bo + bv @ wo.T).

Device formulation (per core, all layouts transposed so no P-transpose is
ever needed):
  QT = (wqT_s.T @ xT + bq)         # [256 qdim, 2048 rows] on chip
  KT = same                        # [256, 2048]
  V  = natural [2048 rows, 256] with an appended ones column per head
  sT[k,q] = sum_d KT[d,k] QT[d,q]  -> psum [128k, 512q] tiles
  mask: additive -1e9 on mixed 128x128 blocks (from the real mask input)
  P = exp(sT/8)  (no max subtraction; scores are O(5) so exp is safe and
      softmax is shift-invariant)
  [OT; l] = [V|1].T @ P            # psum [65, 512q]; row 64 = denominator
  OT_norm = OT * (1/l)             # 1/l replicated over partitions via a
                                   # K=2 indicator matmul, then DVE mult
  yT_partial = woT_s.T @ OT_norm   # [1024, 2048] -> DRAM

All PSUM lives in one pool (tags: big/po/ps1 = 4+2+2 banks) so the Tile
scheduler can overlap projections, attention and output projection.
"""

import os
import hashlib
import numpy as np

B, S, D, H, DK = 2, 2048, 1024, 16, 64
NCORES = 8
GROUPS = 4          # head groups per batch
HPG = 4             # heads per group (per core)
GDIM = HPG * DK     # 256 output dims per core
NEG = -1.0e9
QB = 512            # q block width
NQB = S // QB       # 4
NKT = S // 128      # 16 k tiles
NDM = D // 128      # 8 contraction tiles for projections

MODE = os.environ.get("BASS_MHA_MODE", "bf16")  # fp32 | bf16

_CACHE = {}


def _make_plan(m2d):
    """Classify 128x128 blocks of the (q,k) mask into skip/full/mixed.

    Returns per (qb, j): (j, cmin_local, bias_cols) where bias_cols is a list
    of (c_local, uniq_tile_idx); plus the packed unique bias blocks.
    """
    sub = np.asarray(m2d).reshape(S // 128, 128, S // 128, 128)
    any_ = sub.any(axis=(1, 3))   # [qtile, ktile]
    all_ = sub.all(axis=(1, 3))

    uniq = {}
    uniq_src = []
    plan = []
    for qb in range(NQB):
        entries = []
        cs = list(range(4 * qb, 4 * qb + 4))
        for j in range(NKT):
            states = []
            for c in cs:
                if not any_[c, j]:
                    states.append("skip")
                elif all_[c, j]:
                    states.append("full")
                else:
                    states.append("mixed")
            if all(s == "skip" for s in states):
                continue
            cmin = next(i for i, s in enumerate(states) if s != "skip")
            bias_cols = []
            for i in range(cmin, 4):
                if states[i] == "full":
                    continue
                c = cs[i]
                if states[i] == "skip":
                    blk = np.full((128, 128), NEG, np.float32)
                else:
                    m = sub[c, :, j, :]  # [128 q, 128 k]
                    blk = np.where(m.T != 0, 0.0, NEG).astype(np.float32)
                tri = False  # gpsimd affine_select path regressed; DVE adds
                if tri:
                    # canonical causal diagonal: zero k>q after the exp via
                    # gpsimd affine_select, no bias tile needed
                    bias_cols.append((i, -1))
                    continue
                hsh = hashlib.sha1(blk.tobytes()).hexdigest()
                if hsh not in uniq:
                    uniq[hsh] = len(uniq_src)
                    uniq_src.append(blk)
                bias_cols.append((i, uniq[hsh]))
            entries.append((j, cmin, bias_cols))
        plan.append(entries)
    bias_pack = (
        np.stack(uniq_src) if uniq_src else np.zeros((1, 128, 128), np.float32)
    )
    key = hashlib.sha1(
        repr([(qb, e) for qb, e in enumerate(plan)]).encode()
    ).hexdigest()
    return plan, bias_pack, key


def _build(mode, plan, n_bias):
    import concourse.mybir as mybir
    from concourse import bacc, tile

    f32 = mybir.dt.float32
    bf16 = mybir.dt.bfloat16
    st_dt = bf16 if mode == "bf16" else f32

    AF = mybir.ActivationFunctionType
    AO = mybir.AluOpType

    nc = bacc.Bacc(
        "TRN2", target_bir_lowering=False, debug=False, num_devices=NCORES
    )

    io_dt = bf16 if mode == "bf16" else f32
    qT_d = nc.declare_dram_parameter("qT", [D, S], io_dt, isOutput=False).ap()
    kT_d = nc.declare_dram_parameter("kT", [D, S], io_dt, isOutput=False).ap()
    vT_d = nc.declare_dram_parameter("vT", [D, S], io_dt, isOutput=False).ap()
    wqT_d = nc.declare_dram_parameter("wqT", [D, GDIM], io_dt, isOutput=False).ap()
    wkT_d = nc.declare_dram_parameter("wkT", [D, GDIM], io_dt, isOutput=False).ap()
    wvT_d = nc.declare_dram_parameter("wvT", [D, GDIM], io_dt, isOutput=False).ap()
    woT_d = nc.declare_dram_parameter("woT", [GDIM, D], io_dt, isOutput=False).ap()
    bq_d = nc.declare_dram_parameter("bq2", [128, 2], f32, isOutput=False).ap()
    bk_d = nc.declare_dram_parameter("bk2", [128, 2], f32, isOutput=False).ap()
    bias_d = nc.declare_dram_parameter(
        "bias_pack", [n_bias, 128, 128], f32, isOutput=False
    ).ap()
    yT_d = nc.declare_dram_parameter("yT", [D, S], f32, isOutput=True).ap()

    with tile.TileContext(nc) as tc:
        with (
            tc.tile_pool(name="res", bufs=1) as res,
            tc.tile_pool(name="ot_pool", bufs=2) as ot_pool,
            tc.tile_pool(name="instream", bufs=12) as instream,
            tc.tile_pool(name="ptp", bufs=4) as ptp,
            tc.tile_pool(name="ystage", bufs=4) as ystage,
            tc.tile_pool(name="small", bufs=4) as small,
            tc.tile_pool(name="psum", bufs=2, space="PSUM") as psum,
        ):
            # ---- resident weights / constants ----
            dma = nc.sync.dma_start

            wq_sb = res.tile([128, NDM, GDIM], st_dt, name="wq_sb")
            dma(out=wq_sb, in_=wqT_d.rearrange("(dm p) o -> p dm o", p=128))
            wk_sb = res.tile([128, NDM, GDIM], st_dt, name="wk_sb")
            dma(out=wk_sb, in_=wkT_d.rearrange("(dm p) o -> p dm o", p=128))
            wv_sb = res.tile([128, NDM, GDIM], st_dt, name="wv_sb")
            dma(out=wv_sb, in_=wvT_d.rearrange("(dm p) o -> p dm o", p=128))
            wo_sb = res.tile([128, 2, D], st_dt, name="wo_sb")
            dma(out=wo_sb, in_=woT_d.rearrange("(ct p) o -> p ct o", p=128))
            bq_sb = res.tile([128, 2], f32, name="bq_sb")
            dma(out=bq_sb, in_=bq_d)
            bk_sb = res.tile([128, 2], f32, name="bk_sb")
            dma(out=bk_sb, in_=bk_d)
            bias_sb = res.tile([128, n_bias, 128], f32, name="bias_sb")
            dma(out=bias_sb, in_=bias_d.rearrange("n p o -> p n o"))

            # per-chunk tiles so chunk-level deps stay exact (lets attention
            # on q-block i overlap projections of chunk i+1)
            QT_c = [res.tile([128, 2, 512], st_dt, name=f"QT{i}") for i in range(4)]
            KT_c = [res.tile([128, 2, 512], st_dt, name=f"KT{i}") for i in range(4)]
            V_c = [
                res.tile([128, 4, HPG, DK + 1], st_dt, name=f"V{i}")
                for i in range(4)
            ]
            for i in range(4):
                nc.vector.memset(V_c[i][:, :, :, DK : DK + 1], 1.0)
            ones64_sb = res.tile([1, 64], f32, name="ones64_sb")
            nc.vector.memset(ones64_sb, 1.0)

            def proj_chunk(ci):
                # Q and K projection for 512-row chunk ci
                for src_d, w_sb, b_sb, dst in (
                    (qT_d, wq_sb, bq_sb, QT_c[ci]),
                    (kT_d, wk_sb, bk_sb, KT_c[ci]),
                ):
                    ps_p = psum.tile([128, 2, 512], f32, name="ps_p", tag="big")
                    for dm in range(NDM):
                        xt = instream.tile([128, 512], st_dt, name="xt", tag="xt")
                        dma(
                            out=xt,
                            in_=src_d[
                                128 * dm : 128 * (dm + 1),
                                512 * ci : 512 * (ci + 1),
                            ],
                        )
                        for ot in range(2):
                            nc.tensor.matmul(
                                ps_p[:, ot, :],
                                lhsT=w_sb[:, dm, 128 * ot : 128 * (ot + 1)],
                                rhs=xt,
                                start=(dm == 0),
                                stop=(dm == NDM - 1),
                            )
                    for ot in range(2):
                        nc.vector.tensor_scalar_add(
                            dst[:, ot, :], ps_p[:, ot, :], b_sb[:, ot : ot + 1]
                        )
                # V projection for the same rows
                vts = []
                for dm in range(NDM):
                    vt = instream.tile(
                        [128, 512], st_dt, name="vt", tag="vt", bufs=10
                    )
                    dma(
                        out=vt,
                        in_=vT_d[
                            128 * dm : 128 * (dm + 1),
                            512 * ci : 512 * (ci + 1),
                        ],
                    )
                    vts.append(vt)
                for half in range(2):
                    ps_v = psum.tile([128, 2, 512], f32, name="ps_v", tag="big")
                    for dm in range(NDM):
                        for rl in range(2):
                            rt = 2 * half + rl
                            nc.tensor.matmul(
                                ps_v[:, rl, 0:GDIM],
                                lhsT=vts[dm][:, 128 * rt : 128 * (rt + 1)],
                                rhs=wv_sb[:, dm, :],
                                start=(dm == 0),
                                stop=(dm == NDM - 1),
                            )
                    for rl in range(2):
                        nc.vector.tensor_copy(
                            out=V_c[ci][:, 2 * half + rl, :, 0:DK],
                            in_=ps_v[:, rl, 0:GDIM].rearrange(
                                "p (h d) -> p h d", d=DK
                            ),
                        )

            def attn_pair(qb, pr, OT_sb):
                entries = plan[qb]
                last_j = entries[-1][0]
                first_j = entries[0][0]
                heads = (2 * pr, 2 * pr + 1)
                po = {}
                for h in heads:
                    po[h] = psum.tile(
                        [DK + 1, QB], f32, name=f"po{h}", tag="po", bufs=3
                    )
                for j, cmin, bias_cols in entries:
                    off = 128 * cmin
                    jc, jl = j // 4, j % 4
                    ps_s = psum.tile(
                        [128, 2, QB], f32, name="ps_s", tag="big"
                    )
                    for hh, h in enumerate(heads):
                        p0 = 64 * hh
                        ht = h // 2
                        nc.tensor.matmul(
                            ps_s[:, hh, off:QB],
                            lhsT=KT_c[jc][
                                p0 : p0 + 64, ht, 128 * jl : 128 * (jl + 1)
                            ],
                            rhs=QT_c[qb][p0 : p0 + 64, ht, off:QB],
                            start=True,
                            stop=True,
                        )
                    for hh in range(2):
                        for cl, ui in bias_cols:
                            co = 128 * cl
                            nc.vector.tensor_tensor(
                                out=ps_s[:, hh, co : co + 128],
                                in0=ps_s[:, hh, co : co + 128],
                                in1=bias_sb[:, ui, :],
                                op=AO.add,
                            )
                    pt = ptp.tile([128, 2, QB], st_dt, name="pt")
                    nc.scalar.activation(
                        pt[:, :, off:QB],
                        ps_s[:, :, off:QB],
                        AF.Exp,
                        scale=0.125,
                    )
                    for hh, h in enumerate(heads):
                        nc.tensor.matmul(
                            po[h][:, off:QB],
                            lhsT=V_c[jc][:, jl, h, :],
                            rhs=pt[:, hh, off:QB],
                            start=(j == first_j),
                            stop=(j == last_j),
                        )
                # normalize: 1/l per head, replicate across 64 partitions
                ps_rl = psum.tile(
                    [128, QB], f32, name="ps_rl", tag="ps1", bufs=1
                )
                for hh, h in enumerate(heads):
                    l1 = small.tile([1, QB], f32, name="l1", tag=f"l{hh}")
                    nc.scalar.activation(
                        l1, po[h][DK : DK + 1, :], AF.Copy
                    )
                    rl1 = small.tile(
                        [1, QB], f32, name="rl1", tag=f"rl{hh}"
                    )
                    nc.vector.reciprocal_approx_fast(out=rl1, in_=l1)
                    nc.tensor.matmul(
                        ps_rl[64 * hh : 64 * hh + 64, :],
                        lhsT=ones64_sb,
                        rhs=rl1,
                        start=True,
                        stop=True,
                    )
                rl_bc = small.tile(
                    [128, QB], f32, name="rl_bc", tag="rl_bc"
                )
                nc.scalar.activation(rl_bc, ps_rl, AF.Copy)
                for hh, h in enumerate(heads):
                    p0 = 64 * hh
                    nc.vector.tensor_tensor(
                        out=OT_sb[p0 : p0 + 64, h // 2, :],
                        in0=po[h][0:DK, :],
                        in1=rl_bc[p0 : p0 + 64, :],
                        op=AO.mult,
                    )

            def outproj(qb, OT_sb):
                for ot8 in range(8):
                    ps_y = psum.tile(
                        [128, QB], f32, name="ps_y", tag="ps1", bufs=1
                    )
                    for ct in range(2):
                        nc.tensor.matmul(
                            ps_y,
                            lhsT=wo_sb[:, ct, 128 * ot8 : 128 * (ot8 + 1)],
                            rhs=OT_sb[:, ct, :],
                            start=(ct == 0),
                            stop=(ct == 1),
                        )
                    ysb = ystage.tile([128, QB], f32, name="ysb")
                    nc.vector.tensor_copy(out=ysb, in_=ps_y)
                    dma(
                        out=yT_d[
                            128 * ot8 : 128 * (ot8 + 1),
                            QB * qb : QB * (qb + 1),
                        ],
                        in_=ysb,
                    )

            # projections interleaved with attention; the final pairs of
            # q-blocks 2 and 3 are interleaved so the tail has two
            # independent dependency chains for the scheduler to overlap
            OTm = {}
            for qb in range(4):
                OTm[qb] = ot_pool.tile(
                    [128, 2, QB], st_dt, name=f"OT{qb}", tag="OT", bufs=2
                )
            proj_chunk(0)
            attn_pair(0, 0, OTm[0])
            attn_pair(0, 1, OTm[0])
            outproj(0, OTm[0])
            proj_chunk(1)
            attn_pair(1, 0, OTm[1])
            attn_pair(1, 1, OTm[1])
            outproj(1, OTm[1])
            proj_chunk(2)
            attn_pair(2, 0, OTm[2])
            proj_chunk(3)
            attn_pair(3, 0, OTm[3])
            attn_pair(2, 1, OTm[2])
            outproj(2, OTm[2])
            attn_pair(3, 1, OTm[3])
            outproj(3, OTm[3])


    nc.compile()
    return nc


def _get_nc(mode, plan, n_bias, key):
    ck = (mode, key, n_bias)
    if ck not in _CACHE:
        _CACHE[ck] = _build(mode, plan, n_bias)
    return _CACHE[ck]


def _prep_inputs(q, k, v, wq, bq, wk, bk, wv, wo, bias_pack, mode):
    """Build the 8 per-core input maps."""
    f32 = np.float32
    if mode == "bf16":
        import ml_dtypes

        io_np = ml_dtypes.bfloat16
    else:
        io_np = f32

    wqT = np.ascontiguousarray(np.asarray(wq, f32).T)
    wkT = np.ascontiguousarray(np.asarray(wk, f32).T)
    wvT = np.ascontiguousarray(np.asarray(wv, f32).T)
    woT = np.ascontiguousarray(np.asarray(wo, f32).T)

    in_maps = []
    for c in range(NCORES):
        b, g = c // GROUPS, c % GROUPS
        sl = slice(GDIM * g, GDIM * (g + 1))
        im = {
            "qT": np.ascontiguousarray(np.asarray(q[b], f32).T).astype(io_np),
            "kT": np.ascontiguousarray(np.asarray(k[b], f32).T).astype(io_np),
            "vT": np.ascontiguousarray(np.asarray(v[b], f32).T).astype(io_np),
            "wqT": np.ascontiguousarray(wqT[:, sl]).astype(io_np),
            "wkT": np.ascontiguousarray(wkT[:, sl]).astype(io_np),
            "wvT": np.ascontiguousarray(wvT[:, sl]).astype(io_np),
            "woT": np.ascontiguousarray(woT[sl, :]).astype(io_np),
            "bq2": np.ascontiguousarray(
                np.asarray(bq, f32)[sl].reshape(2, 128).T
            ),
            "bk2": np.ascontiguousarray(
                np.asarray(bk, f32)[sl].reshape(2, 128).T
            ),
            "bias_pack": bias_pack,
        }
        in_maps.append(im)
    return in_maps


def _kernel_impl(q, k, v, mask, wq, bq, wk, bk, wv, bv, wo, bo, trace=False):
    from concourse.bass_utils import run_bass_kernel_spmd

    f32 = np.float32
    m2d = np.asarray(mask)[0, 0]
    plan, bias_pack, key = _make_plan(m2d)
    nc = _get_nc(MODE, plan, bias_pack.shape[0], key)
    in_maps = _prep_inputs(q, k, v, wq, bq, wk, bk, wv, wo, bias_pack, MODE)

    res = run_bass_kernel_spmd(nc, in_maps, list(range(NCORES)), trace=trace)

    bo_eff = (
        np.asarray(bo, np.float64)
        + np.asarray(bv, np.float64) @ np.asarray(wo, np.float64).T
    ).astype(f32)

    out = np.zeros((B, S, D), f32)
    for c in range(NCORES):
        out[c // GROUPS] += res.results[c]["yT"].T
    out += bo_eff
    return out, res


def kernel(q, k, v, mask, wq, bq, wk, bk, wv, bv, wo, bo):
    out, _ = _kernel_impl(q, k, v, mask, wq, bq, wk, bk, wv, bv, wo, bo)
    return out
